# revision 36
# baseline (speedup 1.0000x reference)
"""Trainium2 Bass kernel for GraphTripletGCNLayer (v3).

Reference (N=100000 nodes, R=100000 rels, T=300000 triples, H=256):
    rel = rel_states[rel_idx]
    agg = zeros; agg[obj] += node[subj] + rel; agg[subj] += node[obj] + rel
    out = node + silu(concat([node, agg]) @ W + b)

v3 design (vs v2 window-scheme baseline; ~915us -> ~500us on core 0):
  - Degree-snake dst->(core,group) assignment with a shared in-group position
    shuffle: equalizes per-cell message counts across cores/groups (tight
    SPMD max-over-core tile ceilings) at uniform column density.
  - Messages sorted by dst column within each (group, chunk) cell; tiles are
    full 128-slot cuts of the sorted stream.
  - Per-tile matmul target = exact union column range [c0, c0+rw) across the
    8 cores (program is SPMD-shared, so psum col offsets must be
    core-invariant); regions split only at psum bank (512-col) boundaries.
    Tiles reordered within cells by rw for tight one-hot chunk packing.
  - ALL one-hots of an oh-chunk built with ONE DVE tensor_tensor(is_equal)
    using stride-0 broadcast APs (dcol' fp16 vs iota fp16 -> bf16).
  - Group psum [128, NWG=1536] f32 x2 m-halves, DVE-memset once per group;
    scatter matmuls start=False with skip_group_check.
  - Gathers: per-cell SWDGE calls (<=1024 rows), 4 queues round-robin;
    per-group idx/dcol DMA slices so the first gathers start immediately.
    SWDGE desc-gen on the Pool engine (~3ns/row across 4 queues) is the
    bottleneck at ~95% occupancy; everything else overlaps beneath it.
"""

import os
import sys

sys.path.insert(0, "/opt/trn_rl_repo")

import numpy as np
import ml_dtypes

import concourse.bass as bass
import concourse.bacc as bacc
import concourse.mybir as mybir
import concourse.tile as tile
from concourse.bass import AP
from concourse.bass_utils import run_bass_kernel_spmd

BF16 = mybir.dt.bfloat16
F16 = mybir.dt.float16
F32 = mybir.dt.float32
I16 = mybir.dt.int16

NCORES = 8
WIN = 128                # psum window granularity (cols)
NWG = int(os.environ.get("KNWG", "1536"))   # dst cols per group
BANK = 512               # psum bank cols (f32)
CH = 25000               # table chunk rows (int16 idx limit)
NQ = int(os.environ.get("KNQ", "4"))
SCRATCH = int(os.environ.get("KSCRATCH", "49152"))
GBUFS = int(os.environ.get("KGBUFS", "8"))
OHBUFS = int(os.environ.get("KOHBUFS", "3"))
OH_COLS = int(os.environ.get("KOHCOLS", "4096"))  # max oh chunk width
MAXCALL = 1024           # rows per dma_gather call
USEREG = os.environ.get("KREG", "0") == "1"


def _ceil(a, b):
    return -(-a // b)


def _plan(node_states, rel_states, triples):
    N, H = node_states.shape
    R = rel_states.shape[0]
    assert H == 256, H
    OWN = _ceil(N, NCORES)
    NG = _ceil(OWN, NWG)
    NPAD = NG * NWG
    NWIN = NWG // WIN

    NCH_n = _ceil(N, CH)
    NCH_r = _ceil(R, CH)
    NCHT = NCH_n + NCH_r
    # chunk table: (is_rel, row0, row1)
    chunks = [(0, c * CH, min(N, (c + 1) * CH)) for c in range(NCH_n)] + \
             [(1, c * CH, min(R, (c + 1) * CH)) for c in range(NCH_r)]

    tr = np.asarray(triples).astype(np.int64)
    s, r, o = tr[:, 0], tr[:, 1], tr[:, 2]
    src = np.concatenate([s, o, r + N, r + N])
    dst = np.concatenate([o, s, o, s])

    # degree-snake dst->-(core,pos) assignment: equalizes per-cell counts
    # across cores (tighter SPMD max-over-core tile ceilings / col ranges)
    deg = np.zeros(N, dtype=np.int64)
    np.add.at(deg, o, 1)
    np.add.at(deg, s, 1)
    order = np.argsort(-deg, kind="stable")
    ranks = np.arange(N)
    row8 = ranks // NCORES
    j8 = ranks % NCORES
    coreidx = np.where(row8 % 2 == 0, j8, NCORES - 1 - j8)
    # weighted group interleave: groups 0..NG-2 get R shares each, the LAST
    # group 1 share -> tiny final group -> short post-gather tail. Degrees
    # stay mixed within every group; cross-core rank alignment preserved.
    nrows = int(row8.max()) + 1
    if NG > 1:
        nshare = 1
        for cand in range(7, 0, -1):
            plen = (NG - 1) * cand + 1
            if _ceil(nrows * cand, plen) <= NWG:
                nshare = cand
                break
        PAT = np.concatenate([np.tile(np.arange(NG - 1), nshare), [NG - 1]])
    else:
        PAT = np.array([0])
    g_of_row = PAT[np.arange(nrows) % len(PAT)]
    cntg = np.zeros(NG, dtype=np.int64)
    posg = np.empty(nrows, dtype=np.int64)
    for rr in range(nrows):
        gg = g_of_row[rr]
        posg[rr] = cntg[gg]
        cntg[gg] += 1
    assert cntg.max() <= NWG, (cntg, NWG)
    rngp = np.random.RandomState(12345)
    dloc_of_row = np.empty(nrows, dtype=np.int64)
    for gg in range(NG):
        mrows = g_of_row == gg
        sh = rngp.permutation(int(cntg[gg]))
        dloc_of_row[mrows] = gg * NWG + sh[posg[mrows]]
    posn = dloc_of_row[row8]          # dloc in [0, NPAD)
    assign_core = np.empty(N, dtype=np.int64)
    assign_pos = np.empty(N, dtype=np.int64)
    assign_core[order] = coreidx
    assign_pos[order] = posn
    perm = np.full((NCORES, NPAD), -1, dtype=np.int64)
    perm[coreidx, posn] = order

    core = assign_core[dst]
    dloc = assign_pos[dst]
    g = dloc // NWG
    dgrp = (dloc - g * NWG).astype(np.int32)
    chunk = np.where(src < N, src // CH, NCH_n + (src - N) // CH)
    lidx = np.where(src < N, src % CH, (src - N) % CH).astype(np.int16)
    NCELL = NG * NCHT
    cell = g * NCHT + chunk

    counts = np.zeros((NCORES, NCELL), dtype=np.int64)
    np.add.at(counts, (core, cell), 1)
    Kc = np.maximum(_ceil(counts.max(axis=0), 128), 1)     # tiles per cell
    tile_base = np.concatenate([[0], np.cumsum(Kc)[:-1]])
    T_total = int(Kc.sum())
    S = T_total * 128

    idx_cores = np.zeros((NCORES, S), dtype=np.int16)
    dgrp_cores = np.full((NCORES, S), -1.0, dtype=np.float16)
    a_c = np.full((NCORES, T_total), 1 << 30, dtype=np.int64)  # tile col min
    b_c = np.full((NCORES, T_total), -1, dtype=np.int64)       # tile col max

    for c in range(NCORES):
        m = core == c
        cell_m = cell[m]
        srt = np.lexsort((dgrp[m], cell_m))
        cell_s = cell_m[srt]
        dgrp_s = dgrp[m][srt].astype(np.int64)
        lidx_s = lidx[m][srt]
        starts = np.searchsorted(cell_s, np.arange(NCELL))
        pos = np.arange(cell_s.size) - starts[cell_s]
        slots = tile_base[cell_s] * 128 + pos
        idx_cores[c][slots] = lidx_s
        tos = slots // 128
        np.minimum.at(a_c[c], tos, dgrp_s)
        np.maximum.at(b_c[c], tos, dgrp_s)
        dgrp_cores[c][slots] = dgrp_s.astype(np.float16)   # exact ints < 2048

    c0 = a_c.min(axis=0)
    c1 = b_c.max(axis=0)
    empty = c1 < 0
    c0[empty] = 0
    c1[empty] = 0
    rw = c1 - c0 + 1                 # union col-range width per tile
    # pad width to 16 cols for AP friendliness
    rw = np.minimum((rw + 15) // 16 * 16, NWG - c0)
    assert (c0 + rw).max() <= NWG

    # reorder tiles within each cell by rw desc (tight one-hot chunk packing)
    tperm = np.arange(T_total)
    for ci in range(NCELL):
        t0 = int(tile_base[ci])
        K = int(Kc[ci])
        torder = np.argsort(-rw[t0:t0 + K], kind="stable")
        tperm[t0:t0 + K] = t0 + torder
    c0 = c0[tperm]
    rw = rw[tperm]
    blk_perm_idx = (tperm[:, None] * 128 + np.arange(128)[None, :]).ravel()
    idx_cores = idx_cores[:, blk_perm_idx]
    dgrp_cores = dgrp_cores[:, blk_perm_idx]

    # dcol' = dgrp - c0[tile]  (per slot, -1 stays -1)
    c0_of_slot = np.repeat(c0, 128).astype(np.float16)
    dcol_cores = np.where(dgrp_cores >= 0,
                          dgrp_cores - c0_of_slot,
                          np.float16(-1.0)).astype(np.float16)

    # one-hot chunks: runs of tiles (within a cell, rw-desc sorted) sharing
    # width Wc = rw of first tile; cut at OH_COLS
    oh_chunks = []   # list per cell: list of (t0, nt, Wc)
    for ci in range(NCELL):
        t0 = int(tile_base[ci])
        K = int(Kc[ci])
        out = []
        i = 0
        while i < K:
            Wc = max(int(rw[t0 + i]), 16)
            nt = max(1, min(K - i, OH_COLS // Wc))
            out.append((t0 + i, nt, Wc))
            i += nt
        oh_chunks.append(out)

    idx_wrapped = np.empty((NCORES, 128, S // 16), dtype=np.int16)
    dcol_wrapped = np.empty((NCORES, 128, T_total), dtype=np.float16)
    for c in range(NCORES):
        idx_wrapped[c] = np.tile(idx_cores[c].reshape(-1, 16).T, (8, 1))
        dcol_wrapped[c] = dcol_cores[c].reshape(T_total, 128).T

    # per-call runtime row counts (exact-count gathers skip pad descriptors)
    call_list = []            # (cell, done, nn) in emission order
    for g in range(NG):
        for c in range(NCHT):
            ci = g * NCHT + c
            rows = int(Kc[ci]) * 128
            done = 0
            while done < rows:
                nn = min(MAXCALL, rows - done)
                call_list.append((ci, done, nn))
                done += nn
    NCALLS = len(call_list)
    cnts = np.zeros((NCORES, NCALLS), dtype=np.int32)
    for j, (ci, done, nn) in enumerate(call_list):
        for c in range(NCORES):
            cnt = int(counts[c, ci])
            v = min(max(cnt - done, 16), nn)
            v = min((v + 15) // 16 * 16, nn)
            if os.environ.get("KFULLCNT", "0") == "1":
                v = nn
            cnts[c, j] = v

    RWMAX = int(rw.max())
    return dict(N=N, R=R, H=H, OWN=OWN, NG=NG, NPAD=NPAD, NWIN=NWIN,
                NCHT=NCHT, chunks=chunks, Kc=Kc, tile_base=tile_base,
                T_total=T_total, S=S, c0=c0, rw=rw, RWMAX=RWMAX,
                oh_chunks=oh_chunks, idx=idx_wrapped, dcol=dcol_wrapped,
                call_list=call_list, NCALLS=NCALLS, cnts=cnts, perm=perm)


def _build_program(cfg):
    N, R, H = cfg["N"], cfg["R"], cfg["H"]
    NG, NPAD, NCHT = cfg["NG"], cfg["NPAD"], cfg["NCHT"]
    chunks, Kc, tile_base = cfg["chunks"], cfg["Kc"], cfg["tile_base"]
    T_total, S = cfg["T_total"], cfg["S"]
    c0s, rws = cfg["c0"], cfg["rw"]
    oh_chunks = cfg["oh_chunks"]
    IOTA_W = cfg["RWMAX"]
    KMAX = int(Kc.max())
    CB = 512                      # projection col block

    nc = bacc.Bacc("TRN2", target_bir_lowering=False, debug=False,
                   num_swdge_queues=NQ, dynamic_dma_scratch_size=SCRATCH)

    tab_n = nc.dram_tensor("tab_n", [N, H], BF16, kind="ExternalInput")
    tab_r = nc.dram_tensor("tab_r", [R, H], BF16, kind="ExternalInput")
    idx_d = nc.dram_tensor("idx_d", [128, S // 16], I16, kind="ExternalInput")
    dcol_d = nc.dram_tensor("dcol_d", [128, T_total], F16, kind="ExternalInput")
    NCALLS = cfg["NCALLS"]
    cnts_d = nc.dram_tensor("cnts_d", [1, NCALLS], mybir.dt.int32,
                            kind="ExternalInput")
    widx_d = nc.dram_tensor("widx_d", [128, NQ], I16, kind="ExternalInput")
    ndT16 = nc.dram_tensor("ndT16", [2, 128, NPAD], BF16, kind="ExternalInput")
    w_blk = nc.dram_tensor("w_blk", [128, 8 * 128], BF16, kind="ExternalInput")
    b_blk = nc.dram_tensor("b_blk", [128, 2], F32, kind="ExternalInput")
    iota_d = nc.dram_tensor("iota_d", [128, IOTA_W], F16, kind="ExternalInput")
    yT = nc.dram_tensor("yT", [2, 128, NPAD], BF16, kind="ExternalOutput")

    with tile.TileContext(nc) as tc:
        with (
            tc.tile_pool(name="const", bufs=1) as cpool,
            tc.tile_pool(name="meta", bufs=1) as mpool,
            tc.tile_pool(name="gath", bufs=GBUFS) as gpool,
            tc.tile_pool(name="oh", bufs=OHBUFS) as ohpool,
            tc.tile_pool(name="aggT", bufs=2) as apool,
            tc.tile_pool(name="ndt", bufs=2) as npool,
            tc.tile_pool(name="eout", bufs=2) as epool,
            tc.tile_pool(name="pg", bufs=1, space="PSUM") as pgpool,
            tc.tile_pool(name="psy", bufs=2, space="PSUM") as psy,
        ):
            # warm all SWDGE queues with tiny row-0 gathers while the meta
            # DMAs land: the ~10us first-call ring init then overlaps them
            warm_idx = mpool.tile([128, NQ], I16, tag="widx", name="widx")
            nc.sync.dma_start(warm_idx[:], widx_d[:])
            warm_gt = mpool.tile([128, NQ, H], BF16, tag="wgt", name="wgt")
            for q in range(NQ):
                nc.gpsimd.dma_gather(
                    warm_gt[:, q:q + 1, :], tab_n[0:min(CH, N), :],
                    warm_idx[:, q:q + 1], 16, 16, H, queue_num=q)

            # per-group idx/dcol slices so first gathers start immediately;
            # group 0's slices are the very first DMAs dispatched
            gslice = []       # (tstart, tend) per group
            for g in range(NG):
                tstart = int(tile_base[g * NCHT])
                tend = int(tile_base[g * NCHT + NCHT - 1] +
                           Kc[g * NCHT + NCHT - 1])
                gslice.append((tstart, tend))
            idx_g = []
            dcol_g = []
            for g in range(NG):
                t0g, t1g = gslice[g]
                isb = mpool.tile([128, (t1g - t0g) * 8], I16,
                                 tag=f"idx{g}", name=f"idx{g}")
                nc.sync.dma_start(isb[:], idx_d[:, t0g * 8:t1g * 8])
                idx_g.append(isb)
                dsb = mpool.tile([128, t1g - t0g], F16, tag=f"dc{g}",
                                 name=f"dc{g}")
                nc.sync.dma_start(dsb[:], dcol_d[:, t0g:t1g])
                dcol_g.append(dsb)
            iota_sb = cpool.tile([128, IOTA_W], F16)
            nc.sync.dma_start(iota_sb[:], iota_d[:])
            w_sb = cpool.tile([128, 8 * 128], BF16)
            nc.sync.dma_start(w_sb[:], w_blk[:])
            b_sb = cpool.tile([128, 2], F32)
            nc.sync.dma_start(b_sb[:], b_blk[:])
            cnts_sb = mpool.tile([1, NCALLS], mybir.dt.int32, tag="cnt",
                                 name="cnt")
            nc.sync.dma_start(cnts_sb[:], cnts_d[:])
            if USEREG:
                # value_load (reg_load) is not dependency-tracked; force an
                # in-order Pool read of cnts_sb so later loads see DMA'd data
                cnts_dummy = mpool.tile([1, NCALLS], mybir.dt.int32,
                                        tag="cntd", name="cntd")
                nc.gpsimd.tensor_copy(cnts_dummy[:], cnts_sb[:])

            qctr = 0
            cellctr = 0
            callctr = 0
            for g in range(NG):
                # ---- group psum, zeroed ----
                pw = [pgpool.tile([128, NWG], F32, tag=f"pw{m}", name=f"pw{m}")
                      for m in range(2)]
                for m in range(2):
                    nc.vector.memset(pw[m][:], 0.0)

                # ---- gathers: one-two calls per cell ----
                cells = []
                for c in range(NCHT):
                    ci = g * NCHT + c
                    K = int(Kc[ci])
                    tbase = int(tile_base[ci])
                    is_rel, r0, r1 = chunks[c]
                    tab_t = tab_r if is_rel else tab_n
                    gt = gpool.tile([128, KMAX, H], BF16, tag="g", name="g")
                    # padded gather calls fully overwrite every used tile's
                    # slots (idx pad = row 0), so no NaN-guard memset needed
                    # unless exact-count (USEREG) gathers leave stale slots
                    if USEREG and cellctr < GBUFS:
                        nc.vector.memset(gt[:], 0.0)
                    cellctr += 1
                    rows = K * 128
                    done = 0
                    t0g = gslice[g][0]
                    while done < rows:
                        nn = min(MAXCALL, rows - done)
                        s0 = (tbase - t0g) * 128 + done
                        if USEREG:
                            nreg = nc.gpsimd.value_load(
                                cnts_sb[0:1, callctr:callctr + 1],
                                min_val=16, max_val=nn)
                        else:
                            nreg = nn
                        nc.gpsimd.dma_gather(
                            gt[:, done // 128:(done + nn + 127) // 128, :],
                            tab_t[r0:r1, :],
                            idx_g[g][:, s0 // 16:(s0 + nn) // 16],
                            nn, nreg, H,
                            queue_num=qctr % NQ,
                        )
                        qctr += 1
                        callctr += 1
                        done += nn
                    cells.append((ci, gt, K, tbase))

                # ---- one-hot builds + scatter matmuls ----
                dcol_sb = dcol_g[g]
                t0g = gslice[g][0]
                for (ci, gt, K, tbase) in cells:
                    for (t0, nt, Wc) in oh_chunks[ci]:
                        oh = ohpool.tile([128, OH_COLS], BF16, tag="oh",
                                         name="oh")
                        tl = t0 - t0g
                        dc_ap = AP(dcol_sb.tensor,
                                   dcol_sb[:, tl:tl + 1].offset,
                                   [list(dcol_sb[:].ap[0]), [1, nt], [0, Wc]])
                        io_ap = AP(iota_sb.tensor, iota_sb[:].offset,
                                   [list(iota_sb[:].ap[0]), [0, nt], [1, Wc]])
                        nc.vector.tensor_tensor(
                            oh[:, 0:nt * Wc], dc_ap, io_ap,
                            mybir.AluOpType.is_equal)
                        for i in range(nt):
                            t = t0 + i
                            cc0 = int(c0s[t])
                            cend = cc0 + int(rws[t])
                            # split region at psum bank boundaries
                            bounds = [cc0]
                            nb = (cc0 // BANK + 1) * BANK
                            while nb < cend:
                                bounds.append(nb)
                                nb += BANK
                            bounds.append(cend)
                            tloc = t - tbase
                            for m in range(2):
                                lhsT = gt[:, tloc, m * 128:(m + 1) * 128]
                                for p in range(len(bounds) - 1):
                                    p0, p1 = bounds[p], bounds[p + 1]
                                    o0 = i * Wc + (p0 - cc0)
                                    nc.tensor.matmul(
                                        pw[m][:, p0:p1],
                                        lhsT=lhsT,
                                        rhs=oh[:, o0:o0 + (p1 - p0)],
                                        start=False, stop=False,
                                        skip_group_check=True)

                # ---- drain psum -> aggT (ACT) ----
                aggT = []
                for m in range(2):
                    at = apool.tile([128, NWG], BF16, tag=f"aggT{m}",
                                    name=f"aggT{m}")
                    nc.scalar.activation(at[:], pw[m][:],
                                         mybir.ActivationFunctionType.Copy)
                    aggT.append(at)

                # ---- projection + epilogue ----
                col0 = g * NWG
                nt16 = []
                for m in range(2):
                    t16 = npool.tile([128, NWG], BF16, tag=f"nt16_{m}",
                                     name=f"nt16_{m}")
                    nc.sync.dma_start(t16[:], ndT16[m, :, col0:col0 + NWG])
                    nt16.append(t16)
                for m in range(2):
                    eo = epool.tile([128, NWG], BF16, tag=f"eo{m}",
                                    name=f"eo{m}")
                    for blk in range(NWG // CB):
                        cb0 = blk * CB
                        py = psy.tile([128, CB], F32, tag="py", name="py")
                        for k in range(4):
                            rhs = nt16[k] if k < 2 else aggT[k - 2]
                            kb = k * 2 + m
                            nc.tensor.matmul(
                                py[:],
                                lhsT=w_sb[:, kb * 128:(kb + 1) * 128],
                                rhs=rhs[:, cb0:cb0 + CB],
                                start=(k == 0), stop=(k == 3))
                        nc.scalar.activation(
                            eo[:, cb0:cb0 + CB], py[:],
                            mybir.ActivationFunctionType.Silu,
                            bias=b_sb[:, m:m + 1])
                        nc.vector.tensor_add(
                            eo[:, cb0:cb0 + CB], eo[:, cb0:cb0 + CB],
                            nt16[m][:, cb0:cb0 + CB])
                    nc.sync.dma_start(yT[m, :, col0:col0 + NWG], eo[:])

    nc.finalize()
    return nc


def _host_arrays(cfg, node_states, rel_states, W, b):
    N, H, OWN, NPAD = cfg["N"], cfg["H"], cfg["OWN"], cfg["NPAD"]
    node_states = np.asarray(node_states, dtype=np.float32)
    rel_states = np.asarray(rel_states, dtype=np.float32)
    W = np.asarray(W, dtype=np.float32)
    b = np.asarray(b, dtype=np.float32)

    tab_n = node_states.astype(ml_dtypes.bfloat16)
    tab_r = rel_states.astype(ml_dtypes.bfloat16)
    w_blk = np.zeros((128, 8 * 128), dtype=ml_dtypes.bfloat16)
    for k in range(4):
        for m in range(2):
            kb = k * 2 + m
            w_blk[:, kb * 128:(kb + 1) * 128] = (
                W[k * 128:(k + 1) * 128, m * 128:(m + 1) * 128])
    b_blk = b.reshape(2, 128).T.astype(np.float32).copy()
    IOTA_W = cfg["RWMAX"]
    iota = np.tile(np.arange(IOTA_W, dtype=np.float16)[None, :], (128, 1))

    in_maps = []
    for c in range(NCORES):
        pc = cfg["perm"][c]
        valid = pc >= 0
        slab = np.zeros((NPAD, H), dtype=np.float32)
        slab[np.nonzero(valid)[0]] = node_states[pc[valid]]
        sT = np.ascontiguousarray(slab.T)
        nd16 = sT.reshape(2, 128, NPAD).astype(ml_dtypes.bfloat16)
        im = {
            "tab_n": tab_n, "tab_r": tab_r,
            "idx_d": cfg["idx"][c],
            "dcol_d": cfg["dcol"][c],
            "ndT16": nd16,
            "cnts_d": cfg["cnts"][c][None, :],
            "widx_d": np.zeros((128, NQ), dtype=np.int16),
            "w_blk": w_blk, "b_blk": b_blk, "iota_d": iota,
        }
        in_maps.append(im)
    return in_maps


def kernel(node_states, rel_states, triples, W, b, _trace=False):
    cfg = _plan(np.asarray(node_states), np.asarray(rel_states),
                np.asarray(triples))
    nc = _build_program(cfg)
    in_maps = _host_arrays(cfg, node_states, rel_states, W, b)
    res = run_bass_kernel_spmd(nc, in_maps, core_ids=list(range(NCORES)),
                               trace=_trace)
    N, H, OWN, NPAD = cfg["N"], cfg["H"], cfg["OWN"], cfg["NPAD"]
    out = np.zeros((N, H), dtype=np.float32)
    for c in range(NCORES):
        yTv = res.results[c]["yT"]      # [2, 128, NPAD] bf16
        y = yTv.astype(np.float32).reshape(H, NPAD).T
        pc = cfg["perm"][c]
        valid = pc >= 0
        out[pc[valid]] = y[np.nonzero(valid)[0]]
    if _trace:
        kernel.last_results = res
    return out
